# revision 42
# baseline (speedup 1.0000x reference)
"""Trainium2 Bass kernel for nn_Graph_Enhance_model (GNN message passing).

Self-contained: hardcodes shapes B=4,F=32,H=8,O=16,D=2048, 8 cores.

Phase A (edge waves): data-parallel over the 128 (b,f) frames, 16/core.
  Step-1 wave exploits UM0's structure: its msg_n half is broadcast over h,
  so the wave is a K=1024 matmul plus a rank-64 PSUM update built from
  Q = msg_n @ Wcat[1024:] and the step-0 softmax weights.
Phase B (human GRU): DATA-parallel over frames - each core computes its own
  128 human rows x all 2048 units with the FULL (block-interleaved) GRU
  weights streamed during phase A. M_sum stays core-local: no collectives.
  The h-side gate matmuls run interleaved between phase-A quads.
Phase C (S GRUs): tensor-parallel over the 2048 units, 256/core; needs an
  All_human AllGather (frames-major, transposed on-chip via PE) and an s1
  AllGather, each 64KB.
"""

import os
import sys

for _p in ("/opt/trn_rl_repo", "/opt/pypackages"):
    if _p not in sys.path and os.path.isdir(_p):
        sys.path.append(_p)

import numpy as np
import ml_dtypes

import concourse.bass as bass
import concourse.bacc as bacc
import concourse.tile as tile
import concourse.mybir as mybir
from concourse import bass_utils
from concourse.masks import make_identity

BF16 = mybir.dt.bfloat16
F8 = mybir.dt.float8e4
F32 = mybir.dt.float32
AF = mybir.ActivationFunctionType
ALU = mybir.AluOpType
AX = mybir.AxisListType

NB = ml_dtypes.bfloat16
N8 = ml_dtypes.float8_e4m3

B, F, H, O, D = 4, 32, 8, 16, 2048
NFRAMES = B * F          # 128
NCORES = 8
FPC = NFRAMES // NCORES  # 16 frames per core
ROWS = H * O             # 128 rows per frame
KC = D // 128            # 16 K-chunks
NQ = FPC // 4            # 4 quads of 4 frames
UPC = D // NCORES        # 256 units per core (TP slice, phase C)
GPC = 3 * UPC            # 768 gate columns per core (phase C)

_CACHE = {}
RG = [list(range(NCORES))]
DR = mybir.MatmulPerfMode.DoubleRow


def _build_nc():
    nc = bacc.Bacc("TRN2", target_bir_lowering=False, debug=False, num_devices=NCORES)

    dt_in = {}

    def din(name, shape, dt):
        dt_in[name] = nc.dram_tensor(name, shape, dt, kind="ExternalInput")
        return dt_in[name]

    # per-core phase A (partition-major layouts: contiguous per-partition DMA)
    e0t = din("e0t", [NQ, 128, KC, 512], F8)
    ot = din("ot", [128, KC, FPC * O], BF16)
    wcat = din("wcat", [128, KC, D], F8)
    bl1td = din("bl1t", [128, 8], F32)
    bet0d = din("bet0", [128, 8], F32)
    bet1d = din("bet1", [128, 8], F32)
    wnt = din("wnt", [4, 128, KC, 256], BF16)
    wnb = din("wnb", [1, D // 2], BF16)
    wl2 = din("wl2", [128, 8, 1], BF16)
    scatd = din("scat2", [128, 2, 512], BF16)
    # phase B (DP): full gh GRU weights, block-interleaved [r_j z_j n_j]
    pmatd = din("pmat", [128, FPC], BF16)
    whibd = din("whi_blk", [4, 128, KC, 1536], F8)
    whhbd = din("whh_blk", [4, 128, KC, 1536], F8)
    bhird = din("bhi_r", [1, 3 * D], BF16)
    bhhrd = din("bhh_r", [1, 3 * D], BF16)
    htlocd = din("htloc", [128, KC, 128], F8)
    hlocd = din("hloc", [128, D], BF16)
    # phase C (TP slices + replicated transposed inputs)
    wsid = din("wsi_s", [128, KC, GPC], BF16)
    wshd = din("wsh_s", [128, KC, GPC], BF16)
    bsid = din("bsi_s", [1, GPC], BF16)
    bshd = din("bsh_s", [1, GPC], BF16)
    sc4td = din("sc4t", [128, KC, NFRAMES], BF16)
    sftd = din("sft", [128, KC, NFRAMES], BF16)
    sc4sd = din("sc4_s", [NFRAMES, UPC], F32)
    sfsd = din("sf_s", [NFRAMES, UPC], F32)
    outp = nc.dram_tensor("outp", [NFRAMES, UPC], F32, kind="ExternalOutput")

    from contextlib import ExitStack

    with tile.TileContext(nc) as tc, ExitStack() as ctx:
        glob = ctx.enter_context(tc.tile_pool(name="glob", bufs=1))
        dram = ctx.enter_context(tc.tile_pool(name="dram", bufs=1, space="DRAM"))

        # collective bounce buffers (frames-major ah; unit-major s1)
        bar_cin = dram.tile([1, 16], BF16)
        bar_cout = dram.tile([NCORES, 16], BF16, addr_space="Shared")
        bar2_cin = dram.tile([1, 16], BF16)
        bar2_cout = dram.tile([NCORES, 16], BF16, addr_space="Shared")
        ah_cin = dram.tile([FPC, 4, 512], BF16)
        ah_cout = dram.tile([NCORES * FPC, 4, 512], BF16, addr_space="Shared")
        s1_cin = dram.tile([UPC, NFRAMES], BF16)
        s1_cout = dram.tile([D, NFRAMES], BF16, addr_space="Shared")

        ones_b = glob.tile([1, 512], BF16)
        nc.vector.memset(ones_b, 1.0)
        nc.gpsimd.dma_start(out=bar_cin, in_=ones_b[0:1, 0:16])
        nc.gpsimd.collective_compute(
            "AllGather", ALU.bypass, replica_groups=RG,
            ins=[bar_cin.opt()], outs=[bar_cout.opt()])
        ident128 = glob.tile([128, 128], BF16)
        make_identity(nc, ident128)

        wl2_sb = glob.tile([128, 8, 1], BF16)
        bl1t_sb = glob.tile([128, 8], F32)
        bet0_sb = glob.tile([128, 8], F32)
        bet1_sb = glob.tile([128, 8], F32)
        ones_s0 = glob.tile([1, 128], BF16)
        nc.vector.memset(ones_s0, 0.5)          # w/2 broadcast (step-0 combines)
        ones_s1 = glob.tile([1, 128], BF16)
        nc.vector.memset(ones_s1, 1.0 / 512.0)  # w/512 broadcast (step-1 combines)
        s2048 = glob.tile([128, 1], F32)
        nc.vector.memset(s2048, 1.0 / 32768.0)  # descale for phase-B gi psums (incl /O)
        scat_sb = glob.tile([128, 2, 512], BF16)
        pmat_sb = glob.tile([128, FPC], BF16)

        msgn_sb = glob.tile([128, 8, FPC * O], F8)      # msg_n^T [1024, 256] (for Q)
        msgn_b = glob.tile([128, 8, FPC * O], BF16)     # bf16 copy (vector mn path)
        msum_f = glob.tile([128, KC, 128], BF16)        # M_sum^T local (sum over o)
        msb_all = glob.tile([128, KC, 128], F8)         # f8 x64 copy (phB lhsT)
        ghc = glob.tile([128, 12, 512], BF16)           # parked gh gates (blk*3+g)
        bhir_sb = glob.tile([1, 3 * D], BF16)
        bhhr_sb = glob.tile([1, 3 * D], BF16)

        # phase C weights: wsh/sc4t/sft prefetch on the scalar ring (engine
        # FIFO delays the triggers until after ph0's scalar compute, which
        # keeps them off the critical head window)
        pcw = ctx.enter_context(tc.tile_pool(name="pcw", bufs=1))
        bsi_sb = pcw.tile([1, GPC], BF16)
        bsh_sb = pcw.tile([1, GPC], BF16)
        sc4s_sb = pcw.tile([NFRAMES, UPC], F32)
        sfs_sb = pcw.tile([NFRAMES, UPC], F32)

        def load_phase_c_weights():
            nc.scalar.dma_start(out=bsi_sb, in_=bsid.ap())
            nc.scalar.dma_start(out=bsh_sb, in_=bshd.ap())
            nc.scalar.dma_start(out=sc4s_sb, in_=sc4sd.ap())
            nc.scalar.dma_start(out=sfs_sb, in_=sfsd.ap())

        with tc.tile_pool(name="paq", bufs=1) as paq, \
             tc.tile_pool(name="pbloc", bufs=1) as pbloc, \
             tc.tile_pool(name="pa", bufs=1) as pa, \
             tc.tile_pool(name="pa1", bufs=1) as pa1:
            q_sb = paq.tile([128, 2, D], F8)            # Q for quad-pairs (x32)
            htloc_sb = pbloc.tile([128, KC, 128], F8)   # H_local^T (gh lhsT)
            sgh_c = {}
            for sfx in ("1", "2"):
                sgh_c["rz" + sfx] = pcw.tile([128, 512], F32, name="cgrz" + sfx)
                sgh_c["hn" + sfx] = pcw.tile([128, 256], F32, name="cghn" + sfx)
            if True:
                xq_t = {}

                # ---------------- Phase 0: msg_n^T = Wn @ O^T + bn ----------------
                with nc.named_scope("ph0"):
                    with (
                        tc.tile_pool(name="p0", bufs=1) as p0,
                        tc.tile_pool(name="p0ps", bufs=4, space="PSUM") as p0ps,
                    ):
                        # ring rates: sync ~115GB/s, gpsimd ~35, scalar ~28
                        # (a full scalar ring stalls ACT compute). ph0 feeds
                        # first on sync, wcat follows, bulk weights elsewhere.
                        wnb_sb = p0.tile([1, D // 2], BF16)
                        nc.sync.dma_start(out=wnb_sb, in_=wnb.ap())
                        ot_sb = p0.tile([128, KC, FPC * O], BF16)
                        nc.sync.dma_start(out=ot_sb, in_=ot.ap())
                        wn_t = {}

                        def wn_load(qr, eng):
                            wn_t[qr] = p0.tile([128, KC, 256], BF16, tag="wn", bufs=4,
                                               name=f"wn{qr}")
                            eng.dma_start(out=wn_t[qr], in_=wnt.ap()[qr])

                        wn_load(0, nc.sync)
                        wn_load(1, nc.sync)
                        wn_load(2, nc.gpsimd)
                        wn_load(3, nc.scalar)
                        nc.scalar.dma_start(out=wl2_sb, in_=wl2.ap())
                        nc.scalar.dma_start(out=bl1t_sb, in_=bl1td.ap())
                        nc.scalar.dma_start(out=bet0_sb, in_=bet0d.ap())
                        nc.scalar.dma_start(out=bet1_sb, in_=bet1d.ap())
                        nc.scalar.dma_start(out=scat_sb, in_=scatd.ap())
                        nc.scalar.dma_start(out=pmat_sb, in_=pmatd.ap())
                        nc.scalar.dma_start(out=bhir_sb, in_=bhird.ap())
                        nc.scalar.dma_start(out=bhhr_sb, in_=bhhrd.ap())

                        for quar in range(4):
                            wn_sb = wn_t[quar]
                            for mt2 in range(2):
                                mt = quar * 2 + mt2
                                pm = p0ps.tile([128, FPC * O], F32, tag="pm")
                                for kc in range(KC):
                                    nc.tensor.matmul(pm, lhsT=wn_sb[:, kc, mt2 * 128:(mt2 + 1) * 128],
                                                     rhs=ot_sb[:, kc, :], start=(kc == 0), stop=False)
                                nc.tensor.matmul(pm, lhsT=wnb_sb[0:1, mt * 128:(mt + 1) * 128],
                                                 rhs=ones_b[0:1, 0:FPC * O], start=False, stop=True)
                                nc.scalar.copy(msgn_sb[:, mt, :], pm)
                                nc.scalar.copy(msgn_b[:, mt, :], pm)

                with tc.tile_pool(name="pwhi", bufs=1) as pwhi:
                    # pwhh/pwcat closed manually after the quad loop so their
                    # 56KB frees for phB's temps + phase-C weight staging
                    pwhh_cm = tc.tile_pool(name="pwhh", bufs=1)
                    pwhh = pwhh_cm.__enter__()
                    pwcat_cm = tc.tile_pool(name="pwcat", bufs=1)
                    pwcat = pwcat_cm.__enter__()
                    # gh/gi full-weight blocks stream during phase A. whi on
                    # the fast sync ring (reusing p0's freed space), whh on
                    # gpsimd. xq1 + htloc go ahead of the whi blocks.
                    wcat_sb = pwcat.tile([128, KC, D], F8)
                    nc.sync.dma_start(out=wcat_sb[:, 8:16, :], in_=wcat.ap()[:, 8:16, :])
                    xq_t[0] = pa.tile([128, KC, 512], F8, tag="xq", name="xq0")
                    nc.sync.dma_start(out=xq_t[0], in_=e0t.ap()[0])
                    nc.sync.dma_start(out=wcat_sb[:, 0:8, :], in_=wcat.ap()[:, 0:8, :])
                    xq_t[1] = pa.tile([128, KC, 512], F8, tag="xq", name="xq1")
                    nc.sync.dma_start(out=xq_t[1], in_=e0t.ap()[1])
                    nc.sync.dma_start(out=htloc_sb, in_=htlocd.ap())
                    whi_t = {}
                    for j in range(4):
                        whi_t[j] = pwhi.tile([128, KC, 1536], F8, tag="whib", bufs=2,
                                             name=f"whib{j}")
                        nc.sync.dma_start(out=whi_t[j], in_=whibd.ap()[j])
                    whh_t = {}

                    def whh_load(j):
                        whh_t[j] = pwhh.tile([128, KC, 1536], F8, tag="whhb", bufs=1,
                                             name=f"whhb{j}")
                        nc.gpsimd.dma_start(out=whh_t[j], in_=whhbd.ap()[j])

                    whh_load(0)
                    whh_load(1)
                    load_phase_c_weights()

                    # ------------- Q = msg_n @ Wcat[1024:, :] (step-1 rank update) ----
                    with nc.named_scope("phQ"):
                        with tc.tile_pool(name="pqps", bufs=2, space="PSUM") as pqps:
                            for qq in range(2):
                                for ms in range(4):
                                    pqp = pqps.tile([128, 512], F32, tag="pqp")
                                    for j2 in range(4):
                                        nc.tensor.matmul(pqp,
                                                         lhsT=msgn_sb[:, 2 * j2:2 * j2 + 2, qq * 128:(qq + 1) * 128],
                                                         rhs=wcat_sb[:, 8 + 2 * j2:10 + 2 * j2, ms * 512:(ms + 1) * 512],
                                                         start=(j2 == 0), stop=(j2 == 3),
                                                         perf_mode=DR)
                                    nc.scalar.copy(q_sb[:, qq, ms * 512:(ms + 1) * 512], pqp)

                    # ------------- Phase A: 2 propagation steps over edges -----------
                    with tc.tile_pool(name="paps", bufs=5, space="PSUM") as paps, \
                         tc.tile_pool(name="papss", bufs=1, space="PSUM") as papss:
                        for q in range(NQ):
                            if q >= 2:
                                xq_t[q] = pa.tile([128, KC, 512], F8, tag="xq",
                                                  name=f"xq{q}")
                                nc.sync.dma_start(out=xq_t[q], in_=e0t.ap()[q])
                            xq = xq_t[q]
                            um1t = pa1.tile([128, 8, 512], F8, tag="um1t")
                            wscat = pa1.tile([128, 512], F8, tag="wscat")
                            for step in range(2):
                                early_w = (step == 1 and q == NQ - 1)
                                with nc.named_scope(f"q{q}s{step}"):
                                    def chain(pt, mt):
                                        if step == 0:
                                            # hi-half kc first: wcat-hi lands
                                            # ~25us before wcat-lo on sync
                                            korder = (4, 5, 6, 7, 0, 1, 2, 3)
                                            for ki, k2 in enumerate(korder):
                                                nc.tensor.matmul(pt,
                                                                 lhsT=wcat_sb[:, 2 * k2:2 * k2 + 2, mt * 128:(mt + 1) * 128],
                                                                 rhs=xq[:, 2 * k2:2 * k2 + 2, :],
                                                                 start=(ki == 0), stop=(ki == 7),
                                                                 perf_mode=DR)
                                        else:
                                            for k2 in range(4):
                                                nc.tensor.matmul(pt,
                                                                 lhsT=wcat_sb[:, 2 * k2:2 * k2 + 2, mt * 128:(mt + 1) * 128],
                                                                 rhs=um1t[:, 2 * k2:2 * k2 + 2, :],
                                                                 start=(k2 == 0), stop=False,
                                                                 perf_mode=DR)
                                            nc.tensor.matmul(pt, lhsT=q_sb[:, q // 2, mt * 128:(mt + 1) * 128],
                                                             rhs=wscat, start=False, stop=True)

                                    # --- a-wave: relu(X @ Wl1^T + bl1), transposed ---
                                    psc = 1.0 / 32.0 if step == 0 else 1.0 / 512.0
                                    relu_sb = pa1.tile([128, 8, 512], BF16, tag="relu")
                                    for mt in range(8, 16):
                                        pw_a = paps.tile([128, 512], F32, tag="wave")
                                        chain(pw_a, mt)
                                        nc.scalar.activation(relu_sb[:, mt - 8, :], pw_a, AF.Relu,
                                                             bias=bl1t_sb[:, mt - 8:mt - 7], scale=psc)
                                    # --- logits + softmax over o (groups of 16) ---
                                    pl = papss.tile([1, 512], F32, tag="pl")
                                    for kc2 in range(8):
                                        nc.tensor.matmul(pl, lhsT=wl2_sb[:, kc2, :],
                                                         rhs=relu_sb[:, kc2, :], start=(kc2 == 0), stop=(kc2 == 7))
                                    pl3 = pl.rearrange("o (g i) -> o g i", i=16)
                                    mx = pa1.tile([1, 32], F32, tag="mx")
                                    nc.vector.reduce_max(mx, pl3, axis=AX.X)
                                    sub = pa1.tile([1, 512], F32, tag="sub")
                                    nc.vector.tensor_tensor(sub.rearrange("o (g i) -> o g i", i=16), pl3,
                                                            mx.broadcast_to((1, 32, 16)), op=ALU.subtract)
                                    nc.scalar.activation(sub, sub, AF.Exp)
                                    ex3 = sub.rearrange("o (g i) -> o g i", i=16)
                                    sm = pa1.tile([1, 32], F32, tag="sm")
                                    nc.vector.reduce_sum(sm, ex3, axis=AX.X)
                                    rs = pa1.tile([1, 32], F32, tag="rs")
                                    nc.vector.reciprocal(rs, sm)
                                    w_sb = pa1.tile([1, 512], BF16, tag="w")
                                    nc.vector.tensor_tensor(w_sb.rearrange("o (g i) -> o g i", i=16), ex3,
                                                            rs.broadcast_to((1, 32, 16)), op=ALU.mult)
                                    # --- msg_e wave; w-broadcast MM after 2 groups ---
                                    e_ps = []
                                    wb_sb = pa1.tile([128, 512], F32, tag="wb")
                                    wbs_sb = pa1.tile([128, 512], F32, tag="wbs")

                                    def bcast_w():
                                        pw_b = papss.tile([128, 512], F32, tag="pw")
                                        nc.tensor.matmul(pw_b, lhsT=ones_b[0:1, 0:128], rhs=w_sb,
                                                         start=True, stop=True)
                                        nc.scalar.copy(wb_sb, pw_b)
                                        ssrc = ones_s0 if step == 0 else ones_s1
                                        pw_s = papss.tile([128, 512], F32, tag="pws")
                                        nc.tensor.matmul(pw_s, lhsT=ssrc, rhs=w_sb,
                                                         start=True, stop=True)
                                        nc.scalar.copy(wbs_sb, pw_s)

                                    def msgn_half():
                                        # msg_n half of M_sum: w1-weighted msg_n
                                        # summed over o. Last quad runs the
                                        # multiplies on gpsimd to drain sooner.
                                        wb4 = wb_sb.rearrange("p (f h o) -> p f h o", f=4, h=8)
                                        meng = nc.gpsimd if early_w else nc.vector
                                        for j in range(8):
                                            base = msgn_b[:, j, q * 64:(q + 1) * 64]
                                            mn_bc = bass.AP(tensor=base.tensor, offset=base.offset,
                                                            ap=[list(base.ap[0]), [16, 4], [0, 8], [1, 16]])
                                            tmp = pa1.tile([128, 512], F32, tag=f"um2g{j % 2}" if early_w else "um2")
                                            meng.tensor_tensor(
                                                tmp.rearrange("p (f h o) -> p f h o", f=4, h=8),
                                                mn_bc, wb4, op=ALU.mult)
                                            with nc.allow_low_precision(reason="msum feeds f8 x64 copy"):
                                                nc.vector.reduce_sum(msum_f[:, 8 + j, q * 32:(q + 1) * 32],
                                                                     tmp.rearrange("p (f h o) -> p f h o", f=4, h=8),
                                                                     axis=AX.X)

                                    if early_w:
                                        bcast_w()
                                        msgn_half()

                                    def combine(mt, pe):
                                        if step == 0:
                                            # (32*wave + 32*be) * (w/2) = 16*UM0 -> fp8
                                            nc.vector.scalar_tensor_tensor(
                                                out=um1t[:, mt, :], in0=pe, scalar=bet0_sb[:, mt:mt + 1],
                                                in1=wbs_sb, op0=ALU.add, op1=ALU.mult)
                                        else:
                                            # (512*wave + 512*be) * (w/512) = UM1 exact
                                            tmp = pa1.tile([128, 512], F32, tag="um2")
                                            nc.vector.scalar_tensor_tensor(
                                                out=tmp, in0=pe, scalar=bet1_sb[:, mt:mt + 1],
                                                in1=wbs_sb, op0=ALU.add, op1=ALU.mult)
                                            with nc.allow_low_precision(reason="msum feeds f8 x64 copy"):
                                                nc.vector.reduce_sum(msum_f[:, mt, q * 32:(q + 1) * 32],
                                                                     tmp.rearrange("p (f h o) -> p f h o", f=4, h=8),
                                                                     axis=AX.X)

                                    for mt in range(8):
                                        pe = paps.tile([128, 512], F32, tag="wave")
                                        chain(pe, mt)
                                        e_ps.append(pe)
                                        if mt == 2 and not early_w:
                                            bcast_w()
                                        if mt >= 2:
                                            for cmt in ([0, 1, 2] if mt == 2 else [mt]):
                                                combine(cmt, e_ps[cmt])
                                    if step == 0:
                                        # rank-update rhs for step 1
                                        nc.vector.tensor_tensor(wscat, scat_sb[:, q % 2, :], wb_sb, op=ALU.mult)
                                    elif not early_w:
                                        msgn_half()
                            # M_sum f8 copy for the phB gi lhsT (local, no AG)
                            with nc.named_scope(f"msb{q}"):
                                for kc in range(KC):
                                    nc.scalar.activation(msb_all[:, kc, q * 32:(q + 1) * 32],
                                                         msum_f[:, kc, q * 32:(q + 1) * 32],
                                                         AF.Copy, scale=64.0)
                            # h-side gate block for quad j=q: gh^T-free DP form
                            # out [128 local rows, 512 units] per gate, psums
                            # hold 32*gh (+bias staged x32)
                            with nc.named_scope(f"gh{q}"):
                                for g in range(3):
                                    p_gh = paps.tile([128, 512], F32, tag="wave")
                                    for k2 in range(8):
                                        nc.tensor.matmul(p_gh, lhsT=htloc_sb[:, 2 * k2:2 * k2 + 2, :],
                                                         rhs=whh_t[q][:, 2 * k2:2 * k2 + 2, g * 512:(g + 1) * 512],
                                                         start=(k2 == 0), stop=False, perf_mode=DR)
                                    nc.tensor.matmul(p_gh, lhsT=ones_b[0:1, 0:128],
                                                     rhs=bhhr_sb[0:1, q * 1536 + g * 512:q * 1536 + (g + 1) * 512],
                                                     start=False, stop=True)
                                    nc.scalar.activation(ghc[:, 3 * q + g, :], p_gh, AF.Copy, scale=1.0 / 32.0)
                                if q + 2 < NQ:
                                    whh_load(q + 2)
                            if q == 1:
                                # mid-flight re-sync: absorbs core drift so the
                                # tail ah AG's peer wait stays short
                                nc.scalar.dma_start(out=bar2_cin, in_=ones_b[0:1, 0:16])
                                nc.gpsimd.collective_compute(
                                    "AllGather", ALU.bypass, replica_groups=RG,
                                    ins=[bar2_cin.opt()], outs=[bar2_cout.opt()])

                    pwcat_cm.__exit__(None, None, None)
                    pwhh_cm.__exit__(None, None, None)
                    pcmid_cm = tc.tile_pool(name="pcmid", bufs=1)
                    pcmid = pcmid_cm.__enter__()
                    hloc_sb = pcmid.tile([128, D], BF16)  # H_local rows
                    nc.sync.dma_start(out=hloc_sb, in_=hlocd.ap())
                    sc4t_sb = pcmid.tile([128, KC, NFRAMES], BF16)
                    nc.sync.dma_start(out=sc4t_sb, in_=sc4td.ap())
                    sft_sb = pcmid.tile([128, KC, NFRAMES], BF16)
                    nc.sync.dma_start(out=sft_sb, in_=sftd.ap())

                    # ------------- Phase B: human GRU, DP over frames ----------------
                    # gi chains consume the streamed whi blocks; combine is
                    # elementwise over [128 local rows, 512 units] tiles.
                    with nc.named_scope("phB"):
                        with tc.tile_pool(name="pbps", bufs=6, space="PSUM") as pbps, \
                             tc.tile_pool(name="pahps", bufs=2, space="PSUM") as pahps, \
                             tc.tile_pool(name="pbc", bufs=1) as pbc:
                            ah_sb = pcmid.tile([16, 4, 512], BF16)
                            def gi_chains(j, g):
                                p = pbps.tile([128, 512], F32, tag="pgi", name=f"pgi{j}_{g}")
                                for k2 in range(8):
                                    nc.tensor.matmul(p, lhsT=msb_all[:, 2 * k2:2 * k2 + 2, :],
                                                     rhs=whi_t[j][:, 2 * k2:2 * k2 + 2, g * 512:(g + 1) * 512],
                                                     start=(k2 == 0), stop=False, perf_mode=DR)
                                nc.tensor.matmul(p, lhsT=ones_b[0:1, 0:128],
                                                 rhs=bhir_sb[0:1, j * 1536 + g * 512:j * 1536 + (g + 1) * 512],
                                                 start=False, stop=True)
                                return p

                            # gate-major chain emission (r0..r3, z0..z3, n0..n3)
                            # so the 4-way stage-interleaved combine below can
                            # consume psums early; elementwise stages alternate
                            # vector/gpsimd to halve per-stage latency
                            pg = {}
                            for j in range(4):
                                for g in range(3):
                                    pg[(j, g)] = gi_chains(j, g)
                            JS = range(4)

                            def veng(j):
                                return nc.vector if j < 2 else nc.gpsimd

                            rt, zt, t3, hum = {}, {}, {}, {}
                            for j in JS:
                                rt[j] = pbc.tile([128, 512], F32, tag=f"rt{j}", name=f"rt{j}")
                                nc.vector.scalar_tensor_tensor(
                                    out=rt[j], in0=pg[(j, 0)], scalar=s2048,
                                    in1=ghc[:, 3 * j + 0, :], op0=ALU.mult, op1=ALU.add)
                            for j in JS:
                                nc.scalar.activation(rt[j], rt[j], AF.Sigmoid)
                            for j in JS:
                                zt[j] = pcmid.tile([128, 512], F32, name=f"zt{j}")
                                nc.vector.scalar_tensor_tensor(
                                    out=zt[j], in0=pg[(j, 1)], scalar=s2048,
                                    in1=ghc[:, 3 * j + 1, :], op0=ALU.mult, op1=ALU.add)
                            for j in JS:
                                nc.scalar.activation(zt[j], zt[j], AF.Sigmoid)
                            for j in JS:
                                veng(j).tensor_tensor(rt[j], rt[j], ghc[:, 3 * j + 2, :],
                                                      op=ALU.mult)
                            for j in JS:
                                nc.vector.scalar_tensor_tensor(
                                    out=rt[j], in0=pg[(j, 2)], scalar=s2048,
                                    in1=rt[j], op0=ALU.mult, op1=ALU.add)
                            for j in JS:
                                nc.scalar.activation(rt[j], rt[j], AF.Tanh)  # rt := n
                            for j in JS:
                                t3[j] = pbc.tile([128, 512], F32, tag=f"t3{j & 1}",
                                                 name=f"t3{j}")
                                veng(j).tensor_tensor(t3[j], hloc_sb[:, j * 512:(j + 1) * 512],
                                                      rt[j], op=ALU.subtract)
                                veng(j).tensor_tensor(zt[j], zt[j], t3[j], op=ALU.mult)
                            for j in JS:
                                hum[j] = pcmid.tile([128, 512], BF16, name=f"hum{j}")
                                veng(j).tensor_tensor(hum[j], rt[j], zt[j], op=ALU.add)
                            for j in JS:
                                pah = pahps.tile([16, 512], F32, tag="pah")
                                nc.tensor.matmul(pah, lhsT=pmat_sb, rhs=hum[j], start=True, stop=True)
                                nc.scalar.copy(ah_sb[:, j, :], pah)

                            with nc.named_scope("ah_ag"):
                                nc.scalar.dma_start(out=ah_cin, in_=ah_sb)
                                nc.gpsimd.collective_compute(
                                    "AllGather", ALU.bypass, replica_groups=RG,
                                    ins=[ah_cin.opt()], outs=[ah_cout.opt()])

                    pwsh_cm = tc.tile_pool(name="pwsh", bufs=1)
                    pwsh = pwsh_cm.__enter__()
                    wsh_sb = pwsh.tile([128, KC, GPC], BF16)
                    nc.sync.dma_start(out=wsh_sb, in_=wshd.ap())

                    def s_gh_part2(ht_sb, sfx, pool):
                        with nc.named_scope("phCh" + sfx):
                            p_rz = pool.tile([128, 512], F32, tag="ghp", name="ghp" + sfx)
                            for kc in range(KC):
                                nc.tensor.matmul(p_rz, lhsT=ht_sb[:, kc, :], rhs=wsh_sb[:, kc, 0:512],
                                                 start=(kc == 0), stop=False)
                            nc.tensor.matmul(p_rz, lhsT=ones_b[0:1, 0:128], rhs=bsh_sb[0:1, 0:512],
                                             start=False, stop=True)
                            p_hn = pool.tile([128, 256], F32, tag="ghq", name="ghq" + sfx)
                            for kc in range(KC):
                                nc.tensor.matmul(p_hn, lhsT=ht_sb[:, kc, :], rhs=wsh_sb[:, kc, 512:768],
                                                 start=(kc == 0), stop=False)
                            nc.tensor.matmul(p_hn, lhsT=ones_b[0:1, 0:128], rhs=bsh_sb[0:1, 512:768],
                                             start=False, stop=True)
                            nc.scalar.copy(sgh_c["rz" + sfx], p_rz)
                            nc.scalar.copy(sgh_c["hn" + sfx], p_hn)

                    with tc.tile_pool(name="pcghp", bufs=2, space="PSUM") as pcghp:
                        s_gh_part2(sc4t_sb, "1", pcghp)
                        s_gh_part2(sft_sb, "2", pcghp)
                    pwsh_cm.__exit__(None, None, None)
                    pcmid_cm.__exit__(None, None, None)

        # ---------------- Phase C: two S-node GRUs, TP over units ----------------
        with tc.tile_pool(name="pcgh", bufs=1) as pcgh, \
             tc.tile_pool(name="pcw2", bufs=1) as pcw2:
            # wsi lands in freed phase-A space; sync ring is free by now
            wsh_sb = pcw2.tile([128, KC, GPC], BF16)
            nc.sync.dma_start(out=wsh_sb, in_=wshd.ap())
            sc4t_sb = pcw2.tile([128, KC, NFRAMES], BF16)
            nc.sync.dma_start(out=sc4t_sb, in_=sc4td.ap())
            sft_sb = pcw2.tile([128, KC, NFRAMES], BF16)
            nc.sync.dma_start(out=sft_sb, in_=sftd.ap())
            wsi_sb = pcw2.tile([128, KC, GPC], BF16)
            nc.sync.dma_start(out=wsi_sb[:, 0:8, :], in_=wsid.ap()[:, 0:8, :])
            nc.gpsimd.dma_start(out=wsi_sb[:, 8:16, :], in_=wsid.ap()[:, 8:16, :])

            def s_gh_part(ht_sb, sfx, pool):
                with nc.named_scope("phCh" + sfx):
                    p_rz = pool.tile([128, 512], F32, tag="ghp", name="ghp" + sfx)
                    for kc in range(KC):
                        nc.tensor.matmul(p_rz, lhsT=ht_sb[:, kc, :], rhs=wsh_sb[:, kc, 0:512],
                                         start=(kc == 0), stop=False)
                    nc.tensor.matmul(p_rz, lhsT=ones_b[0:1, 0:128], rhs=bsh_sb[0:1, 0:512],
                                     start=False, stop=True)
                    p_hn = pool.tile([128, 256], F32, tag="ghq", name="ghq" + sfx)
                    for kc in range(KC):
                        nc.tensor.matmul(p_hn, lhsT=ht_sb[:, kc, :], rhs=wsh_sb[:, kc, 512:768],
                                         start=(kc == 0), stop=False)
                    nc.tensor.matmul(p_hn, lhsT=ones_b[0:1, 0:128], rhs=bsh_sb[0:1, 512:768],
                                     start=False, stop=True)
                    grz_c = pcgh.tile([128, 512], F32, tag="cgrz" + sfx, name="cgrz" + sfx)
                    nc.scalar.copy(grz_c, p_rz)
                    ghn_c = pcgh.tile([128, 256], F32, tag="cghn" + sfx, name="cghn" + sfx)
                    nc.scalar.copy(ghn_c, p_hn)
                    return grz_c, ghn_c

            # phase C h-side chains fill the PE window while the ah AG flies
            with tc.tile_pool(name="pcghp", bufs=2, space="PSUM") as pcghp:
                gh_rz1, gh_hn1 = s_gh_part(sc4t_sb, "1", pcghp)
                gh_rz2, gh_hn2 = s_gh_part(sft_sb, "2", pcghp)

            with (
                tc.tile_pool(name="pc1", bufs=1) as pc1,
                tc.tile_pool(name="pcsm", bufs=1) as pcsm,
                tc.tile_pool(name="pcps", bufs=1, space="PSUM") as pcps,
                tc.tile_pool(name="pctps", bufs=2, space="PSUM") as pctps,
            ):
                def s_gi_part(xt_sb, sfx):
                    with nc.named_scope("phCx" + sfx):
                        p_rz = pcps.tile([128, 512], F32, tag="sgz" + sfx, name="sgz" + sfx)
                        for kc in range(KC):
                            nc.tensor.matmul(p_rz, lhsT=xt_sb[:, kc, :], rhs=wsi_sb[:, kc, 0:512],
                                             start=(kc == 0), stop=False)
                        nc.tensor.matmul(p_rz, lhsT=ones_b[0:1, 0:128], rhs=bsi_sb[0:1, 0:512],
                                         start=False, stop=True)
                        p_in = pcps.tile([128, 256], F32, tag="sin" + sfx, name="sin" + sfx)
                        for kc in range(KC):
                            nc.tensor.matmul(p_in, lhsT=xt_sb[:, kc, :], rhs=wsi_sb[:, kc, 512:768],
                                             start=(kc == 0), stop=False)
                        nc.tensor.matmul(p_in, lhsT=ones_b[0:1, 0:128], rhs=bsi_sb[0:1, 512:768],
                                         start=False, stop=True)
                        return p_rz, p_in

                def s_gru_elem(p_giz, gh_rz, p_in, gh_hn, h_sb, out_sb):
                    grs = pcsm.tile([128, 512], F32, tag="grs")
                    nc.vector.tensor_tensor(grs, p_giz, gh_rz, op=ALU.add)
                    rz = pcsm.tile([128, 512], F32, tag="crz")
                    nc.scalar.activation(rz, grs, AF.Sigmoid)
                    u1 = pcsm.tile([128, 256], F32, tag="u1")
                    nc.vector.tensor_tensor(u1, rz[:, 0:256], gh_hn, op=ALU.mult)
                    u2 = pcsm.tile([128, 256], F32, tag="u2")
                    nc.vector.tensor_tensor(u2, u1, p_in, op=ALU.add)
                    n1 = pcsm.tile([128, 256], F32, tag="n1")
                    nc.scalar.activation(n1, u2, AF.Tanh)
                    u3 = pcsm.tile([128, 256], F32, tag="u3")
                    nc.vector.tensor_tensor(u3, h_sb, n1, op=ALU.subtract)
                    u4 = pcsm.tile([128, 256], F32, tag="u4")
                    nc.vector.tensor_tensor(u4, rz[:, 256:512], u3, op=ALU.mult)
                    nc.vector.tensor_tensor(out_sb, n1, u4, op=ALU.add)

                # gathered All_human [128 global frames, 2048]: load frames-
                # major (contiguous) then PE-transpose into lhsT chunk form
                ah_fr = pc1.tile([128, 4, 512], BF16)
                nc.sync.dma_start(out=ah_fr, in_=ah_cout)
                aht = pc1.tile([128, KC, 128], BF16)
                af2 = ah_fr.rearrange("f j (c n) -> f (j c) n", n=128)
                for kc in range(KC):
                    ptp = pctps.tile([128, 128], BF16, tag="tp")
                    nc.tensor.transpose(ptp, af2[:, kc, :], ident128)
                    nc.scalar.copy(aht[:, kc, :], ptp)
                p_giz1, p_in1 = s_gi_part(aht, "1")
                s1_b = pc1.tile([NFRAMES, UPC], BF16)
                s_gru_elem(p_giz1, gh_rz1, p_in1, gh_hn1, sc4s_sb, s1_b)
                # transpose s1 slice -> [units, frames], gather to full s1^T
                s1T = pc1.tile([128, 2, 128], BF16)
                for u2 in range(2):
                    ptp = pctps.tile([128, 128], BF16, tag="tp")
                    nc.tensor.transpose(ptp, s1_b[:, u2 * 128:(u2 + 1) * 128], ident128)
                    nc.scalar.copy(s1T[:, u2, :], ptp)
                with nc.named_scope("s1_ag"):
                    nc.scalar.dma_start(out=s1_cin.rearrange("(c p) n -> p c n", p=128), in_=s1T)
                    nc.gpsimd.collective_compute(
                        "AllGather", ALU.bypass, replica_groups=RG,
                        ins=[s1_cin.opt()], outs=[s1_cout.opt()])
                s1t_all = pc1.tile([128, KC, 128], BF16)
                nc.sync.dma_start(out=s1t_all, in_=s1_cout.rearrange("(kc p) n -> p kc n", p=128))
                p_giz2, p_in2 = s_gi_part(s1t_all, "2")
                out_sb = pc1.tile([NFRAMES, UPC], F32)
                s_gru_elem(p_giz2, gh_rz2, p_in2, gh_hn2, sfs_sb, out_sb)
                nc.sync.dma_start(out=outp.ap(), in_=out_sb)

    nc.compile()
    return nc


def _pm(a):
    """[KC'*128, N] -> partition-major [128, KC', N] contiguous."""
    rows, n = a.shape
    kc = rows // 128
    return np.ascontiguousarray(a.reshape(kc, 128, n).transpose(1, 0, 2))


def _blk_interleave(Wt):
    """[D, 3*D] gate-major -> [4, 128, KC, 1536] block-interleaved pm."""
    blocks = []
    for j in range(4):
        cols = [Wt[:, g * D + j * 512:g * D + (j + 1) * 512] for g in range(3)]
        blocks.append(_pm(np.ascontiguousarray(np.concatenate(cols, axis=1))))
    return np.stack(blocks)


def _blk_bias(b, scale):
    parts = [b[g * D + j * 512:g * D + (j + 1) * 512] for j in range(4) for g in range(3)]
    return (np.concatenate(parts)[None, :] * scale).astype(NB)


def _prep_in_maps(inputs):
    E = np.ascontiguousarray(inputs["H_O_edges"].reshape(NFRAMES, ROWS, D))
    On = inputs["O_nodes"].reshape(NFRAMES, O, D)
    Hn = inputs["H_nodes"].reshape(NFRAMES, H, D)
    Sc4 = inputs["S_node_C4"].reshape(NFRAMES, D)
    Sf = np.ascontiguousarray(inputs["final_S_node"].transpose(0, 2, 1)).reshape(NFRAMES, D)

    whi_t = np.ascontiguousarray(inputs["gh_wih"].T)
    whh_t = np.ascontiguousarray(inputs["gh_whh"].T)
    wsi_t = np.ascontiguousarray(inputs["gs_wih"].T)
    wsh_t = np.ascontiguousarray(inputs["gs_whh"].T)

    def slice_gates(Wt, c):
        return np.ascontiguousarray(np.concatenate(
            [Wt[:, g * D + c * UPC:g * D + (c + 1) * UPC] for g in range(3)], axis=1))

    def slice_bias(b, c):
        return np.concatenate(
            [b[g * D + c * UPC:g * D + (c + 1) * UPC] for g in range(3)])[None, :]

    # scat2[:, par, :]: maps quad-column (f,h,o) to Q-pair row (f + 4*par, o)
    # value 16 so wscat = scat*w = 16*w matches the fp8 scale plan
    scat = np.zeros((128, 2, 512), np.float32)
    for par in range(2):
        for f in range(4):
            for h in range(H):
                for o in range(O):
                    scat[(f + 4 * par) * O + o, par, f * 128 + h * O + o] = 16.0

    wnt_full = np.ascontiguousarray(inputs["Wn"].T).astype(NB)  # [D, 1024]
    wnt_q = np.stack([_pm(wnt_full[:, qr * 256:(qr + 1) * 256]) for qr in range(4)])

    shared = {
        "wcat": _pm(np.clip(np.ascontiguousarray(
            np.concatenate([inputs["We"], inputs["Wl1"]], axis=0).T) * 32.0,
            -240, 240).astype(N8)),
        "bl1t": np.ascontiguousarray(inputs["bl1"].reshape(8, 128).T).astype(np.float32),
        "bet0": np.ascontiguousarray(32.0 * inputs["be"].reshape(8, 128).T).astype(np.float32),
        "bet1": np.ascontiguousarray(512.0 * inputs["be"].reshape(8, 128).T).astype(np.float32),
        "pmat": np.ascontiguousarray(np.kron(np.eye(FPC), np.ones((H, 1))) / H).astype(NB),
        "wnt": wnt_q,
        "wnb": inputs["bn"][None, :].astype(NB),
        "wl2": np.ascontiguousarray(inputs["Wl2"][0].reshape(8, 128).T)[:, :, None].astype(NB),
        "scat2": scat.astype(NB),
        "whi_blk": np.clip(_blk_interleave(whi_t) * 32.0, -240, 240).astype(N8),
        "whh_blk": np.clip(_blk_interleave(whh_t) * 32.0, -240, 240).astype(N8),
        "bhi_r": _blk_bias(inputs["gh_bih"], 32768.0),
        "bhh_r": _blk_bias(inputs["gh_bhh"], 32.0),
        "sc4t": _pm(np.ascontiguousarray(Sc4.T).astype(NB)),
        "sft": _pm(np.ascontiguousarray(Sf.T).astype(NB)),
    }

    in_maps = []
    for c in range(NCORES):
        fr = slice(c * FPC, (c + 1) * FPC)
        us = slice(c * UPC, (c + 1) * UPC)
        Ec = E[fr]  # [16, 128, 2048]
        e0t = np.clip(np.ascontiguousarray(
            Ec.reshape(NQ, 4, ROWS, D).transpose(0, 3, 1, 2).reshape(NQ, D, 512)), -240, 240).astype(N8)
        e0t = np.ascontiguousarray(
            e0t.reshape(NQ, KC, 128, 512).transpose(0, 2, 1, 3))  # [NQ,128,KC,512]
        ot = _pm(np.ascontiguousarray(On[fr].reshape(FPC * O, D).T).astype(NB))
        Hl = Hn[fr].reshape(FPC * H, D)  # [128 local rows, D], row=(f,h)
        m = dict(shared)
        m.update({
            "e0t": e0t,
            "ot": ot,
            "htloc": _pm(np.clip(np.ascontiguousarray(Hl.T), -240, 240).astype(N8)),
            "hloc": Hl.astype(NB),
            "wsi_s": _pm(slice_gates(wsi_t, c).astype(NB)),
            "wsh_s": _pm(slice_gates(wsh_t, c).astype(NB)),
            "bsi_s": slice_bias(inputs["gs_bih"], c).astype(NB),
            "bsh_s": slice_bias(inputs["gs_bhh"], c).astype(NB),
            "sc4_s": np.ascontiguousarray(Sc4[:, us]).astype(np.float32),
            "sf_s": np.ascontiguousarray(Sf[:, us]).astype(np.float32),
        })
        in_maps.append(m)
    return in_maps


LAST_RESULT = None


def kernel(**inputs):
    global LAST_RESULT
    if "nc" not in _CACHE:
        _CACHE["nc"] = _build_nc()
    nc = _CACHE["nc"]
    in_maps = _prep_in_maps(inputs)
    trace = os.environ.get("KERNEL_TRACE", "0") == "1"
    res = bass_utils.run_bass_kernel_spmd(
        nc, in_maps, core_ids=list(range(NCORES)), trace=trace)
    LAST_RESULT = res
    out = np.concatenate([res.results[c]["outp"] for c in range(NCORES)], axis=1)
    return np.ascontiguousarray(out.reshape(B, F, D)).astype(np.float32)


if __name__ == "__main__":
    np.random.seed(0)
    ins = {
        "S_node_C4": np.random.randn(B, F, D).astype(np.float32),
        "final_S_node": np.random.randn(B, D, F).astype(np.float32),
        "H_nodes": np.random.randn(B, F, H, D).astype(np.float32),
        "O_nodes": np.random.randn(B, F, O, D).astype(np.float32),
        "H_O_edges": np.random.randn(B, F, H, O, D).astype(np.float32),
        "Wn": np.random.randn(D // 2, D).astype(np.float32) * 0.02,
        "bn": np.random.randn(D // 2).astype(np.float32) * 0.02,
        "We": np.random.randn(D // 2, D).astype(np.float32) * 0.02,
        "be": np.random.randn(D // 2).astype(np.float32) * 0.02,
        "Wl1": np.random.randn(D // 2, D).astype(np.float32) * 0.02,
        "bl1": np.random.randn(D // 2).astype(np.float32) * 0.02,
        "Wl2": np.random.randn(1, D // 2).astype(np.float32) * 0.02,
        "bl2": np.random.randn(1).astype(np.float32) * 0.02,
        "gh_wih": np.random.randn(3 * D, D).astype(np.float32) * 0.02,
        "gh_whh": np.random.randn(3 * D, D).astype(np.float32) * 0.02,
        "gh_bih": np.random.randn(3 * D).astype(np.float32) * 0.02,
        "gh_bhh": np.random.randn(3 * D).astype(np.float32) * 0.02,
        "gs_wih": np.random.randn(3 * D, D).astype(np.float32) * 0.02,
        "gs_whh": np.random.randn(3 * D, D).astype(np.float32) * 0.02,
        "gs_bih": np.random.randn(3 * D).astype(np.float32) * 0.02,
        "gs_bhh": np.random.randn(3 * D).astype(np.float32) * 0.02,
    }
    out = kernel(**ins)
    print("kernel ran, out shape", out.shape, out.dtype, float(np.abs(out).mean()))


# revision 43
# speedup vs baseline: 1.3385x; 1.3385x over previous
"""Trainium2 Bass kernel for nn_Graph_Enhance_model (GNN message passing).

Self-contained: hardcodes shapes B=4,F=32,H=8,O=16,D=2048, 8 cores.

Phase A (edge waves): data-parallel over the 128 (b,f) frames, 16/core.
  Step-1 wave exploits UM0's structure: its msg_n half is broadcast over h,
  so the wave is a K=1024 matmul plus a rank-64 PSUM update built from
  Q = msg_n @ Wcat[1024:] and the step-0 softmax weights.
Phase B (human GRU): DATA-parallel over frames - each core computes its own
  128 human rows x all 2048 units with the FULL (block-interleaved) GRU
  weights streamed during phase A. M_sum stays core-local: no collectives.
  The h-side gate matmuls run interleaved between phase-A quads.
Phase C (S GRUs): tensor-parallel over the 2048 units, 256/core; needs an
  All_human AllGather (frames-major, transposed on-chip via PE) and an s1
  AllGather, each 64KB.
"""

import os
import sys

for _p in ("/opt/trn_rl_repo", "/opt/pypackages"):
    if _p not in sys.path and os.path.isdir(_p):
        sys.path.append(_p)

import numpy as np
import ml_dtypes

import concourse.bass as bass
import concourse.bacc as bacc
import concourse.tile as tile
import concourse.mybir as mybir
from concourse import bass_utils
from concourse.masks import make_identity

BF16 = mybir.dt.bfloat16
F8 = mybir.dt.float8e4
F32 = mybir.dt.float32
AF = mybir.ActivationFunctionType
ALU = mybir.AluOpType
AX = mybir.AxisListType

NB = ml_dtypes.bfloat16
N8 = ml_dtypes.float8_e4m3

B, F, H, O, D = 4, 32, 8, 16, 2048
NFRAMES = B * F          # 128
NCORES = 8
FPC = NFRAMES // NCORES  # 16 frames per core
ROWS = H * O             # 128 rows per frame
KC = D // 128            # 16 K-chunks
NQ = FPC // 4            # 4 quads of 4 frames
UPC = D // NCORES        # 256 units per core (TP slice, phase C)
GPC = 3 * UPC            # 768 gate columns per core (phase C)

_CACHE = {}
RG = [list(range(NCORES))]
DR = mybir.MatmulPerfMode.DoubleRow


def _build_nc():
    nc = bacc.Bacc("TRN2", target_bir_lowering=False, debug=False, num_devices=NCORES)

    dt_in = {}

    def din(name, shape, dt):
        dt_in[name] = nc.dram_tensor(name, shape, dt, kind="ExternalInput")
        return dt_in[name]

    # per-core phase A (partition-major layouts: contiguous per-partition DMA)
    e0t = din("e0t", [NQ, 128, KC, 512], F8)
    ot = din("ot", [128, KC, FPC * O], BF16)
    wcat = din("wcat", [128, KC, D], F8)
    bl1td = din("bl1t", [128, 8], F32)
    bet0d = din("bet0", [128, 8], F32)
    bet1d = din("bet1", [128, 8], F32)
    wnt = din("wnt", [4, 128, KC, 256], BF16)
    wnb = din("wnb", [1, D // 2], BF16)
    wl2 = din("wl2", [128, 8, 1], BF16)
    scatd = din("scat2", [128, 2, 512], BF16)
    # phase B (DP): full gh GRU weights, block-interleaved [r_j z_j n_j]
    pmatd = din("pmat", [128, FPC], BF16)
    whibd = din("whi_blk", [4, 128, KC, 1536], F8)
    whhbd = din("whh_blk", [4, 128, KC, 1536], F8)
    bhird = din("bhi_r", [1, 3 * D], BF16)
    bhhrd = din("bhh_r", [1, 3 * D], BF16)
    htlocd = din("htloc", [128, KC, 128], F8)
    hlocd = din("hloc", [128, D], BF16)
    # phase C (TP slices + replicated transposed inputs)
    wsid = din("wsi_s", [128, KC, GPC], BF16)
    wshd = din("wsh_s", [128, KC, GPC], BF16)
    bsid = din("bsi_s", [1, GPC], BF16)
    bshd = din("bsh_s", [1, GPC], BF16)
    sc4td = din("sc4t", [128, KC, NFRAMES], BF16)
    sftd = din("sft", [128, KC, NFRAMES], BF16)
    sc4sd = din("sc4_s", [NFRAMES, UPC], F32)
    sfsd = din("sf_s", [NFRAMES, UPC], F32)
    outp = nc.dram_tensor("outp", [NFRAMES, UPC], F32, kind="ExternalOutput")

    from contextlib import ExitStack

    with tile.TileContext(nc) as tc, ExitStack() as ctx:
        glob = ctx.enter_context(tc.tile_pool(name="glob", bufs=1))
        dram = ctx.enter_context(tc.tile_pool(name="dram", bufs=1, space="DRAM"))

        # collective bounce buffers (frames-major ah; unit-major s1)
        bar_cin = dram.tile([1, 16], BF16)
        bar_cout = dram.tile([NCORES, 16], BF16, addr_space="Shared")
        bar2_cin = dram.tile([1, 16], BF16)
        bar2_cout = dram.tile([NCORES, 16], BF16, addr_space="Shared")
        ah_cin = dram.tile([FPC, 4, 512], BF16)
        ah_cout = dram.tile([NCORES * FPC, 4, 512], BF16, addr_space="Shared")
        s1_cin = dram.tile([UPC, NFRAMES], BF16)
        s1_cout = dram.tile([D, NFRAMES], BF16, addr_space="Shared")

        ones_b = glob.tile([1, 512], BF16)
        nc.vector.memset(ones_b, 1.0)
        nc.gpsimd.dma_start(out=bar_cin, in_=ones_b[0:1, 0:16])
        nc.gpsimd.collective_compute(
            "AllGather", ALU.bypass, replica_groups=RG,
            ins=[bar_cin.opt()], outs=[bar_cout.opt()])
        ident128 = glob.tile([128, 128], BF16)
        make_identity(nc, ident128)

        wl2_sb = glob.tile([128, 8, 1], BF16)
        bl1t_sb = glob.tile([128, 8], F32)
        bet0_sb = glob.tile([128, 8], F32)
        bet1_sb = glob.tile([128, 8], F32)
        ones_s0 = glob.tile([1, 128], BF16)
        nc.vector.memset(ones_s0, 0.5)          # w/2 broadcast (step-0 combines)
        ones_s1 = glob.tile([1, 128], BF16)
        nc.vector.memset(ones_s1, 1.0 / 512.0)  # w/512 broadcast (step-1 combines)
        s2048 = glob.tile([128, 1], F32)
        nc.vector.memset(s2048, 1.0 / 32768.0)  # descale for phase-B gi psums (incl /O)
        scat_sb = glob.tile([128, 2, 512], BF16)
        pmat_sb = glob.tile([128, FPC], BF16)

        msgn_sb = glob.tile([128, 8, FPC * O], F8)      # msg_n^T [1024, 256] (for Q)
        msgn_b = glob.tile([128, 8, FPC * O], BF16)     # bf16 copy (vector mn path)
        msum_f = glob.tile([128, KC, 128], BF16)        # M_sum^T local (sum over o)
        msb_all = glob.tile([128, KC, 128], F8)         # f8 x64 copy (phB lhsT)
        ghc = glob.tile([128, 12, 512], BF16)           # parked gh gates (blk*3+g)
        bhir_sb = glob.tile([1, 3 * D], BF16)
        bhhr_sb = glob.tile([1, 3 * D], BF16)

        # phase C weights: wsh/sc4t/sft prefetch on the scalar ring (engine
        # FIFO delays the triggers until after ph0's scalar compute, which
        # keeps them off the critical head window)
        pcw = ctx.enter_context(tc.tile_pool(name="pcw", bufs=1))
        bsi_sb = pcw.tile([1, GPC], BF16)
        bsh_sb = pcw.tile([1, GPC], BF16)
        sc4s_sb = pcw.tile([NFRAMES, UPC], F32)
        sfs_sb = pcw.tile([NFRAMES, UPC], F32)

        def load_phase_c_weights():
            nc.scalar.dma_start(out=bsi_sb, in_=bsid.ap())
            nc.scalar.dma_start(out=bsh_sb, in_=bshd.ap())
            nc.scalar.dma_start(out=sc4s_sb, in_=sc4sd.ap())
            nc.scalar.dma_start(out=sfs_sb, in_=sfsd.ap())

        with tc.tile_pool(name="paq", bufs=1) as paq, \
             tc.tile_pool(name="pbloc", bufs=1) as pbloc, \
             tc.tile_pool(name="pa", bufs=1) as pa, \
             tc.tile_pool(name="pa1", bufs=1) as pa1:
            q_sb = paq.tile([128, 2, D], F8)            # Q for quad-pairs (x32)
            htloc_sb = pbloc.tile([128, KC, 128], F8)   # H_local^T (gh lhsT)
            sgh_c = {}
            for sfx in ("1", "2"):
                sgh_c["rz" + sfx] = pcw.tile([128, 512], F32, name="cgrz" + sfx)
                sgh_c["hn" + sfx] = pcw.tile([128, 256], F32, name="cghn" + sfx)
            if True:
                xq_t = {}

                # ---------------- Phase 0: msg_n^T = Wn @ O^T + bn ----------------
                with nc.named_scope("ph0"):
                    with (
                        tc.tile_pool(name="p0", bufs=1) as p0,
                        tc.tile_pool(name="p0ps", bufs=4, space="PSUM") as p0ps,
                    ):
                        # ring rates: sync ~115GB/s, gpsimd ~35, scalar ~28
                        # (a full scalar ring stalls ACT compute). ph0 feeds
                        # first on sync, wcat follows, bulk weights elsewhere.
                        wnb_sb = p0.tile([1, D // 2], BF16)
                        nc.sync.dma_start(out=wnb_sb, in_=wnb.ap())
                        ot_sb = p0.tile([128, KC, FPC * O], BF16)
                        nc.sync.dma_start(out=ot_sb, in_=ot.ap())
                        wn_t = {}

                        def wn_load(qr, eng):
                            wn_t[qr] = p0.tile([128, KC, 256], BF16, tag="wn", bufs=4,
                                               name=f"wn{qr}")
                            eng.dma_start(out=wn_t[qr], in_=wnt.ap()[qr])

                        wn_load(0, nc.sync)
                        wn_load(1, nc.sync)
                        wn_load(2, nc.gpsimd)
                        wn_load(3, nc.scalar)
                        nc.scalar.dma_start(out=wl2_sb, in_=wl2.ap())
                        nc.scalar.dma_start(out=bl1t_sb, in_=bl1td.ap())
                        nc.scalar.dma_start(out=bet0_sb, in_=bet0d.ap())
                        nc.scalar.dma_start(out=bet1_sb, in_=bet1d.ap())
                        nc.scalar.dma_start(out=scat_sb, in_=scatd.ap())
                        nc.scalar.dma_start(out=pmat_sb, in_=pmatd.ap())
                        nc.scalar.dma_start(out=bhir_sb, in_=bhird.ap())
                        nc.scalar.dma_start(out=bhhr_sb, in_=bhhrd.ap())

                        for quar in range(4):
                            wn_sb = wn_t[quar]
                            for mt2 in range(2):
                                mt = quar * 2 + mt2
                                pm = p0ps.tile([128, FPC * O], F32, tag="pm")
                                for kc in range(KC):
                                    nc.tensor.matmul(pm, lhsT=wn_sb[:, kc, mt2 * 128:(mt2 + 1) * 128],
                                                     rhs=ot_sb[:, kc, :], start=(kc == 0), stop=False)
                                nc.tensor.matmul(pm, lhsT=wnb_sb[0:1, mt * 128:(mt + 1) * 128],
                                                 rhs=ones_b[0:1, 0:FPC * O], start=False, stop=True)
                                nc.scalar.copy(msgn_sb[:, mt, :], pm)
                                nc.scalar.copy(msgn_b[:, mt, :], pm)

                with tc.tile_pool(name="pwhi", bufs=1) as pwhi:
                    # pwhh/pwcat closed manually after the quad loop so their
                    # 56KB frees for phB's temps + phase-C weight staging
                    pwhh_cm = tc.tile_pool(name="pwhh", bufs=1)
                    pwhh = pwhh_cm.__enter__()
                    pwcat_cm = tc.tile_pool(name="pwcat", bufs=1)
                    pwcat = pwcat_cm.__enter__()
                    # gh/gi full-weight blocks stream during phase A. whi on
                    # the fast sync ring (reusing p0's freed space), whh on
                    # gpsimd. xq1 + htloc go ahead of the whi blocks.
                    wcat_sb = pwcat.tile([128, KC, D], F8)
                    nc.sync.dma_start(out=wcat_sb[:, 8:16, :], in_=wcat.ap()[:, 8:16, :])
                    xq_t[0] = pa.tile([128, KC, 512], F8, tag="xq", name="xq0")
                    nc.sync.dma_start(out=xq_t[0], in_=e0t.ap()[0])
                    nc.sync.dma_start(out=wcat_sb[:, 0:8, :], in_=wcat.ap()[:, 0:8, :])
                    xq_t[1] = pa.tile([128, KC, 512], F8, tag="xq", name="xq1")
                    nc.sync.dma_start(out=xq_t[1], in_=e0t.ap()[1])
                    nc.sync.dma_start(out=htloc_sb, in_=htlocd.ap())
                    whi_t = {}
                    for j in range(4):
                        whi_t[j] = pwhi.tile([128, KC, 1536], F8, tag="whib", bufs=2,
                                             name=f"whib{j}")
                        nc.sync.dma_start(out=whi_t[j], in_=whibd.ap()[j])
                    whh_t = {}

                    def whh_load(j):
                        whh_t[j] = pwhh.tile([128, KC, 1536], F8, tag="whhb", bufs=1,
                                             name=f"whhb{j}")
                        nc.gpsimd.dma_start(out=whh_t[j], in_=whhbd.ap()[j])

                    whh_load(0)
                    whh_load(1)
                    load_phase_c_weights()

                    # ------------- Q = msg_n @ Wcat[1024:, :] (step-1 rank update) ----
                    with nc.named_scope("phQ"):
                        with tc.tile_pool(name="pqps", bufs=2, space="PSUM") as pqps:
                            for qq in range(2):
                                for ms in range(4):
                                    pqp = pqps.tile([128, 512], F32, tag="pqp")
                                    for j2 in range(4):
                                        nc.tensor.matmul(pqp,
                                                         lhsT=msgn_sb[:, 2 * j2:2 * j2 + 2, qq * 128:(qq + 1) * 128],
                                                         rhs=wcat_sb[:, 8 + 2 * j2:10 + 2 * j2, ms * 512:(ms + 1) * 512],
                                                         start=(j2 == 0), stop=(j2 == 3),
                                                         perf_mode=DR)
                                    nc.scalar.copy(q_sb[:, qq, ms * 512:(ms + 1) * 512], pqp)

                    # ------------- Phase A: 2 propagation steps over edges -----------
                    with tc.tile_pool(name="paps", bufs=5, space="PSUM") as paps, \
                         tc.tile_pool(name="papss", bufs=1, space="PSUM") as papss:
                        for q in range(NQ):
                            if q >= 2:
                                xq_t[q] = pa.tile([128, KC, 512], F8, tag="xq",
                                                  name=f"xq{q}")
                                nc.sync.dma_start(out=xq_t[q], in_=e0t.ap()[q])
                            xq = xq_t[q]
                            um1t = pa1.tile([128, 8, 512], F8, tag="um1t")
                            wscat = pa1.tile([128, 512], F8, tag="wscat")
                            for step in range(2):
                                early_w = (step == 1 and q == NQ - 1)
                                with nc.named_scope(f"q{q}s{step}"):
                                    def chain(pt, mt):
                                        if step == 0:
                                            # hi-half kc first: wcat-hi lands
                                            # ~25us before wcat-lo on sync
                                            korder = (4, 5, 6, 7, 0, 1, 2, 3)
                                            for ki, k2 in enumerate(korder):
                                                nc.tensor.matmul(pt,
                                                                 lhsT=wcat_sb[:, 2 * k2:2 * k2 + 2, mt * 128:(mt + 1) * 128],
                                                                 rhs=xq[:, 2 * k2:2 * k2 + 2, :],
                                                                 start=(ki == 0), stop=(ki == 7),
                                                                 perf_mode=DR)
                                        else:
                                            for k2 in range(4):
                                                nc.tensor.matmul(pt,
                                                                 lhsT=wcat_sb[:, 2 * k2:2 * k2 + 2, mt * 128:(mt + 1) * 128],
                                                                 rhs=um1t[:, 2 * k2:2 * k2 + 2, :],
                                                                 start=(k2 == 0), stop=False,
                                                                 perf_mode=DR)
                                            nc.tensor.matmul(pt, lhsT=q_sb[:, q // 2, mt * 128:(mt + 1) * 128],
                                                             rhs=wscat, start=False, stop=True)

                                    # --- a-wave: relu(X @ Wl1^T + bl1), transposed ---
                                    psc = 1.0 / 32.0 if step == 0 else 1.0 / 512.0
                                    relu_sb = pa1.tile([128, 8, 512], BF16, tag="relu")
                                    for mt in range(8, 16):
                                        pw_a = paps.tile([128, 512], F32, tag="wave")
                                        chain(pw_a, mt)
                                        nc.scalar.activation(relu_sb[:, mt - 8, :], pw_a, AF.Relu,
                                                             bias=bl1t_sb[:, mt - 8:mt - 7], scale=psc)
                                    # --- logits + softmax over o (groups of 16) ---
                                    pl = papss.tile([1, 512], F32, tag="pl")
                                    for kc2 in range(8):
                                        nc.tensor.matmul(pl, lhsT=wl2_sb[:, kc2, :],
                                                         rhs=relu_sb[:, kc2, :], start=(kc2 == 0), stop=(kc2 == 7))
                                    pl3 = pl.rearrange("o (g i) -> o g i", i=16)
                                    mx = pa1.tile([1, 32], F32, tag="mx")
                                    nc.vector.reduce_max(mx, pl3, axis=AX.X)
                                    sub = pa1.tile([1, 512], F32, tag="sub")
                                    nc.vector.tensor_tensor(sub.rearrange("o (g i) -> o g i", i=16), pl3,
                                                            mx.broadcast_to((1, 32, 16)), op=ALU.subtract)
                                    nc.scalar.activation(sub, sub, AF.Exp)
                                    ex3 = sub.rearrange("o (g i) -> o g i", i=16)
                                    sm = pa1.tile([1, 32], F32, tag="sm")
                                    nc.vector.reduce_sum(sm, ex3, axis=AX.X)
                                    rs = pa1.tile([1, 32], F32, tag="rs")
                                    nc.vector.reciprocal(rs, sm)
                                    w_sb = pa1.tile([1, 512], BF16, tag="w")
                                    nc.vector.tensor_tensor(w_sb.rearrange("o (g i) -> o g i", i=16), ex3,
                                                            rs.broadcast_to((1, 32, 16)), op=ALU.mult)
                                    # --- msg_e wave; w-broadcast MM after 2 groups ---
                                    e_ps = []
                                    wb_sb = pa1.tile([128, 512], F32, tag="wb")
                                    wbs_sb = pa1.tile([128, 512], F32, tag="wbs")

                                    def bcast_w():
                                        pw_b = papss.tile([128, 512], F32, tag="pw")
                                        nc.tensor.matmul(pw_b, lhsT=ones_b[0:1, 0:128], rhs=w_sb,
                                                         start=True, stop=True)
                                        nc.scalar.copy(wb_sb, pw_b)
                                        ssrc = ones_s0 if step == 0 else ones_s1
                                        pw_s = papss.tile([128, 512], F32, tag="pws")
                                        nc.tensor.matmul(pw_s, lhsT=ssrc, rhs=w_sb,
                                                         start=True, stop=True)
                                        nc.scalar.copy(wbs_sb, pw_s)

                                    def msgn_half():
                                        # msg_n half of M_sum: w1-weighted msg_n
                                        # summed over o. Last quad runs the
                                        # multiplies on gpsimd to drain sooner.
                                        wb4 = wb_sb.rearrange("p (f h o) -> p f h o", f=4, h=8)
                                        meng = nc.gpsimd if early_w else nc.vector
                                        for j in range(8):
                                            base = msgn_b[:, j, q * 64:(q + 1) * 64]
                                            mn_bc = bass.AP(tensor=base.tensor, offset=base.offset,
                                                            ap=[list(base.ap[0]), [16, 4], [0, 8], [1, 16]])
                                            tmp = pa1.tile([128, 512], F32, tag=f"um2g{j % 2}" if early_w else "um2")
                                            meng.tensor_tensor(
                                                tmp.rearrange("p (f h o) -> p f h o", f=4, h=8),
                                                mn_bc, wb4, op=ALU.mult)
                                            with nc.allow_low_precision(reason="msum feeds f8 x64 copy"):
                                                nc.vector.reduce_sum(msum_f[:, 8 + j, q * 32:(q + 1) * 32],
                                                                     tmp.rearrange("p (f h o) -> p f h o", f=4, h=8),
                                                                     axis=AX.X)

                                    if early_w:
                                        bcast_w()
                                        msgn_half()

                                    def combine(mt, pe):
                                        if step == 0:
                                            # (32*wave + 32*be) * (w/2) = 16*UM0 -> fp8
                                            nc.vector.scalar_tensor_tensor(
                                                out=um1t[:, mt, :], in0=pe, scalar=bet0_sb[:, mt:mt + 1],
                                                in1=wbs_sb, op0=ALU.add, op1=ALU.mult)
                                        else:
                                            # (512*wave + 512*be) * (w/512) = UM1 exact
                                            tmp = pa1.tile([128, 512], F32, tag="um2")
                                            nc.vector.scalar_tensor_tensor(
                                                out=tmp, in0=pe, scalar=bet1_sb[:, mt:mt + 1],
                                                in1=wbs_sb, op0=ALU.add, op1=ALU.mult)
                                            with nc.allow_low_precision(reason="msum feeds f8 x64 copy"):
                                                nc.vector.reduce_sum(msum_f[:, mt, q * 32:(q + 1) * 32],
                                                                     tmp.rearrange("p (f h o) -> p f h o", f=4, h=8),
                                                                     axis=AX.X)

                                    for mt in range(8):
                                        pe = paps.tile([128, 512], F32, tag="wave")
                                        chain(pe, mt)
                                        e_ps.append(pe)
                                        if mt == 2 and not early_w:
                                            bcast_w()
                                        if mt >= 2:
                                            for cmt in ([0, 1, 2] if mt == 2 else [mt]):
                                                combine(cmt, e_ps[cmt])
                                    if step == 0:
                                        # rank-update rhs for step 1
                                        nc.vector.tensor_tensor(wscat, scat_sb[:, q % 2, :], wb_sb, op=ALU.mult)
                                    elif not early_w:
                                        msgn_half()
                            # M_sum f8 copy for the phB gi lhsT (local, no AG)
                            with nc.named_scope(f"msb{q}"):
                                for kc in range(KC):
                                    nc.scalar.activation(msb_all[:, kc, q * 32:(q + 1) * 32],
                                                         msum_f[:, kc, q * 32:(q + 1) * 32],
                                                         AF.Copy, scale=64.0)
                            # h-side gate block for quad j=q: gh^T-free DP form
                            # out [128 local rows, 512 units] per gate, psums
                            # hold 32*gh (+bias staged x32)
                            with nc.named_scope(f"gh{q}"):
                                for g in range(3):
                                    p_gh = paps.tile([128, 512], F32, tag="wave")
                                    for k2 in range(8):
                                        nc.tensor.matmul(p_gh, lhsT=htloc_sb[:, 2 * k2:2 * k2 + 2, :],
                                                         rhs=whh_t[q][:, 2 * k2:2 * k2 + 2, g * 512:(g + 1) * 512],
                                                         start=(k2 == 0), stop=False, perf_mode=DR)
                                    nc.tensor.matmul(p_gh, lhsT=ones_b[0:1, 0:128],
                                                     rhs=bhhr_sb[0:1, q * 1536 + g * 512:q * 1536 + (g + 1) * 512],
                                                     start=False, stop=True)
                                    nc.scalar.activation(ghc[:, 3 * q + g, :], p_gh, AF.Copy, scale=1.0 / 32.0)
                                if q + 2 < NQ:
                                    whh_load(q + 2)
                            if q == 1:
                                # mid-flight re-sync: absorbs core drift so the
                                # tail ah AG's peer wait stays short
                                nc.scalar.dma_start(out=bar2_cin, in_=ones_b[0:1, 0:16])
                                nc.gpsimd.collective_compute(
                                    "AllGather", ALU.bypass, replica_groups=RG,
                                    ins=[bar2_cin.opt()], outs=[bar2_cout.opt()])

                    pwcat_cm.__exit__(None, None, None)
                    pwhh_cm.__exit__(None, None, None)
                    pcmid_cm = tc.tile_pool(name="pcmid", bufs=1)
                    pcmid = pcmid_cm.__enter__()
                    hloc_sb = pcmid.tile([128, D], BF16)  # H_local rows
                    nc.sync.dma_start(out=hloc_sb, in_=hlocd.ap())
                    sc4t_sb = pcmid.tile([128, KC, NFRAMES], BF16)
                    nc.sync.dma_start(out=sc4t_sb, in_=sc4td.ap())
                    sft_sb = pcmid.tile([128, KC, NFRAMES], BF16)
                    nc.sync.dma_start(out=sft_sb, in_=sftd.ap())

                    # ------------- Phase B: human GRU, DP over frames ----------------
                    # gi chains consume the streamed whi blocks; combine is
                    # elementwise over [128 local rows, 512 units] tiles.
                    with nc.named_scope("phB"):
                        with tc.tile_pool(name="pbps", bufs=6, space="PSUM") as pbps, \
                             tc.tile_pool(name="pahps", bufs=2, space="PSUM") as pahps, \
                             tc.tile_pool(name="pbc", bufs=1) as pbc:
                            ah_sb = pcmid.tile([16, 4, 512], BF16)
                            def gi_chains(j, g):
                                p = pbps.tile([128, 512], F32, tag="pgi", name=f"pgi{j}_{g}")
                                for k2 in range(8):
                                    nc.tensor.matmul(p, lhsT=msb_all[:, 2 * k2:2 * k2 + 2, :],
                                                     rhs=whi_t[j][:, 2 * k2:2 * k2 + 2, g * 512:(g + 1) * 512],
                                                     start=(k2 == 0), stop=False, perf_mode=DR)
                                nc.tensor.matmul(p, lhsT=ones_b[0:1, 0:128],
                                                 rhs=bhir_sb[0:1, j * 1536 + g * 512:j * 1536 + (g + 1) * 512],
                                                 start=False, stop=True)
                                return p

                            # gate-major chain emission (r0..r3, z0..z3, n0..n3)
                            # so the 4-way stage-interleaved combine below can
                            # consume psums early; elementwise stages alternate
                            # vector/gpsimd to halve per-stage latency
                            pg = {}
                            for j in range(4):
                                for g in range(3):
                                    pg[(j, g)] = gi_chains(j, g)
                            JS = range(4)

                            def veng(j):
                                return nc.vector

                            rt, zt, t3, hum = {}, {}, {}, {}
                            for j in JS:
                                rt[j] = pbc.tile([128, 512], F32, tag=f"rt{j}", name=f"rt{j}")
                                nc.vector.scalar_tensor_tensor(
                                    out=rt[j], in0=pg[(j, 0)], scalar=s2048,
                                    in1=ghc[:, 3 * j + 0, :], op0=ALU.mult, op1=ALU.add)
                            for j in JS:
                                nc.scalar.activation(rt[j], rt[j], AF.Sigmoid)
                            for j in JS:
                                zt[j] = pcmid.tile([128, 512], F32, name=f"zt{j}")
                                nc.vector.scalar_tensor_tensor(
                                    out=zt[j], in0=pg[(j, 1)], scalar=s2048,
                                    in1=ghc[:, 3 * j + 1, :], op0=ALU.mult, op1=ALU.add)
                            for j in JS:
                                nc.scalar.activation(zt[j], zt[j], AF.Sigmoid)
                            for j in JS:
                                veng(j).tensor_tensor(rt[j], rt[j], ghc[:, 3 * j + 2, :],
                                                      op=ALU.mult)
                            for j in JS:
                                nc.vector.scalar_tensor_tensor(
                                    out=rt[j], in0=pg[(j, 2)], scalar=s2048,
                                    in1=rt[j], op0=ALU.mult, op1=ALU.add)
                            for j in JS:
                                nc.scalar.activation(rt[j], rt[j], AF.Tanh)  # rt := n
                            for j in JS:
                                t3[j] = pbc.tile([128, 512], F32, tag=f"t3{j & 1}",
                                                 name=f"t3{j}")
                                veng(j).tensor_tensor(t3[j], hloc_sb[:, j * 512:(j + 1) * 512],
                                                      rt[j], op=ALU.subtract)
                                veng(j).tensor_tensor(zt[j], zt[j], t3[j], op=ALU.mult)
                            for j in JS:
                                hum[j] = pcmid.tile([128, 512], BF16, name=f"hum{j}")
                                veng(j).tensor_tensor(hum[j], rt[j], zt[j], op=ALU.add)
                            for j in JS:
                                pah = pahps.tile([16, 512], F32, tag="pah")
                                nc.tensor.matmul(pah, lhsT=pmat_sb, rhs=hum[j], start=True, stop=True)
                                nc.scalar.copy(ah_sb[:, j, :], pah)

                            with nc.named_scope("ah_ag"):
                                nc.scalar.dma_start(out=ah_cin, in_=ah_sb)
                                nc.gpsimd.collective_compute(
                                    "AllGather", ALU.bypass, replica_groups=RG,
                                    ins=[ah_cin.opt()], outs=[ah_cout.opt()])

                    pwsh_cm = tc.tile_pool(name="pwsh", bufs=1)
                    pwsh = pwsh_cm.__enter__()
                    wsh_sb = pwsh.tile([128, KC, GPC], BF16)
                    nc.sync.dma_start(out=wsh_sb, in_=wshd.ap())

                    def s_gh_part2(ht_sb, sfx, pool):
                        with nc.named_scope("phCh" + sfx):
                            p_rz = pool.tile([128, 512], F32, tag="ghp", name="ghp" + sfx)
                            for kc in range(KC):
                                nc.tensor.matmul(p_rz, lhsT=ht_sb[:, kc, :], rhs=wsh_sb[:, kc, 0:512],
                                                 start=(kc == 0), stop=False)
                            nc.tensor.matmul(p_rz, lhsT=ones_b[0:1, 0:128], rhs=bsh_sb[0:1, 0:512],
                                             start=False, stop=True)
                            p_hn = pool.tile([128, 256], F32, tag="ghq", name="ghq" + sfx)
                            for kc in range(KC):
                                nc.tensor.matmul(p_hn, lhsT=ht_sb[:, kc, :], rhs=wsh_sb[:, kc, 512:768],
                                                 start=(kc == 0), stop=False)
                            nc.tensor.matmul(p_hn, lhsT=ones_b[0:1, 0:128], rhs=bsh_sb[0:1, 512:768],
                                             start=False, stop=True)
                            nc.scalar.copy(sgh_c["rz" + sfx], p_rz)
                            nc.scalar.copy(sgh_c["hn" + sfx], p_hn)

                    with tc.tile_pool(name="pcghp", bufs=2, space="PSUM") as pcghp:
                        s_gh_part2(sc4t_sb, "1", pcghp)
                        s_gh_part2(sft_sb, "2", pcghp)
                    pwsh_cm.__exit__(None, None, None)
                    pcmid_cm.__exit__(None, None, None)

        # ---------------- Phase C: two S-node GRUs, TP over units ----------------
        with tc.tile_pool(name="pcgh", bufs=1) as pcgh, \
             tc.tile_pool(name="pcw2", bufs=1) as pcw2:
            # wsi lands in freed phase-A space; sync ring is free by now
            wsh_sb = pcw2.tile([128, KC, GPC], BF16)
            nc.sync.dma_start(out=wsh_sb, in_=wshd.ap())
            sc4t_sb = pcw2.tile([128, KC, NFRAMES], BF16)
            nc.sync.dma_start(out=sc4t_sb, in_=sc4td.ap())
            sft_sb = pcw2.tile([128, KC, NFRAMES], BF16)
            nc.sync.dma_start(out=sft_sb, in_=sftd.ap())
            wsi_sb = pcw2.tile([128, KC, GPC], BF16)
            nc.sync.dma_start(out=wsi_sb[:, 0:8, :], in_=wsid.ap()[:, 0:8, :])
            nc.gpsimd.dma_start(out=wsi_sb[:, 8:16, :], in_=wsid.ap()[:, 8:16, :])

            def s_gh_part(ht_sb, sfx, pool):
                with nc.named_scope("phCh" + sfx):
                    p_rz = pool.tile([128, 512], F32, tag="ghp", name="ghp" + sfx)
                    for kc in range(KC):
                        nc.tensor.matmul(p_rz, lhsT=ht_sb[:, kc, :], rhs=wsh_sb[:, kc, 0:512],
                                         start=(kc == 0), stop=False)
                    nc.tensor.matmul(p_rz, lhsT=ones_b[0:1, 0:128], rhs=bsh_sb[0:1, 0:512],
                                     start=False, stop=True)
                    p_hn = pool.tile([128, 256], F32, tag="ghq", name="ghq" + sfx)
                    for kc in range(KC):
                        nc.tensor.matmul(p_hn, lhsT=ht_sb[:, kc, :], rhs=wsh_sb[:, kc, 512:768],
                                         start=(kc == 0), stop=False)
                    nc.tensor.matmul(p_hn, lhsT=ones_b[0:1, 0:128], rhs=bsh_sb[0:1, 512:768],
                                     start=False, stop=True)
                    grz_c = pcgh.tile([128, 512], F32, tag="cgrz" + sfx, name="cgrz" + sfx)
                    nc.scalar.copy(grz_c, p_rz)
                    ghn_c = pcgh.tile([128, 256], F32, tag="cghn" + sfx, name="cghn" + sfx)
                    nc.scalar.copy(ghn_c, p_hn)
                    return grz_c, ghn_c

            # phase C h-side chains fill the PE window while the ah AG flies
            with tc.tile_pool(name="pcghp", bufs=2, space="PSUM") as pcghp:
                gh_rz1, gh_hn1 = s_gh_part(sc4t_sb, "1", pcghp)
                gh_rz2, gh_hn2 = s_gh_part(sft_sb, "2", pcghp)

            with (
                tc.tile_pool(name="pc1", bufs=1) as pc1,
                tc.tile_pool(name="pcsm", bufs=1) as pcsm,
                tc.tile_pool(name="pcps", bufs=1, space="PSUM") as pcps,
                tc.tile_pool(name="pctps", bufs=2, space="PSUM") as pctps,
            ):
                def s_gi_part(xt_sb, sfx):
                    with nc.named_scope("phCx" + sfx):
                        p_rz = pcps.tile([128, 512], F32, tag="sgz" + sfx, name="sgz" + sfx)
                        for kc in range(KC):
                            nc.tensor.matmul(p_rz, lhsT=xt_sb[:, kc, :], rhs=wsi_sb[:, kc, 0:512],
                                             start=(kc == 0), stop=False)
                        nc.tensor.matmul(p_rz, lhsT=ones_b[0:1, 0:128], rhs=bsi_sb[0:1, 0:512],
                                         start=False, stop=True)
                        p_in = pcps.tile([128, 256], F32, tag="sin" + sfx, name="sin" + sfx)
                        for kc in range(KC):
                            nc.tensor.matmul(p_in, lhsT=xt_sb[:, kc, :], rhs=wsi_sb[:, kc, 512:768],
                                             start=(kc == 0), stop=False)
                        nc.tensor.matmul(p_in, lhsT=ones_b[0:1, 0:128], rhs=bsi_sb[0:1, 512:768],
                                         start=False, stop=True)
                        return p_rz, p_in

                def s_gru_elem(p_giz, gh_rz, p_in, gh_hn, h_sb, out_sb):
                    grs = pcsm.tile([128, 512], F32, tag="grs")
                    nc.vector.tensor_tensor(grs, p_giz, gh_rz, op=ALU.add)
                    rz = pcsm.tile([128, 512], F32, tag="crz")
                    nc.scalar.activation(rz, grs, AF.Sigmoid)
                    u1 = pcsm.tile([128, 256], F32, tag="u1")
                    nc.vector.tensor_tensor(u1, rz[:, 0:256], gh_hn, op=ALU.mult)
                    u2 = pcsm.tile([128, 256], F32, tag="u2")
                    nc.vector.tensor_tensor(u2, u1, p_in, op=ALU.add)
                    n1 = pcsm.tile([128, 256], F32, tag="n1")
                    nc.scalar.activation(n1, u2, AF.Tanh)
                    u3 = pcsm.tile([128, 256], F32, tag="u3")
                    nc.vector.tensor_tensor(u3, h_sb, n1, op=ALU.subtract)
                    u4 = pcsm.tile([128, 256], F32, tag="u4")
                    nc.vector.tensor_tensor(u4, rz[:, 256:512], u3, op=ALU.mult)
                    nc.vector.tensor_tensor(out_sb, n1, u4, op=ALU.add)

                # gathered All_human [128 global frames, 2048]: load frames-
                # major (contiguous) then PE-transpose into lhsT chunk form
                ah_fr = pc1.tile([128, 4, 512], BF16)
                nc.sync.dma_start(out=ah_fr, in_=ah_cout)
                aht = pc1.tile([128, KC, 128], BF16)
                af2 = ah_fr.rearrange("f j (c n) -> f (j c) n", n=128)
                for kc in range(KC):
                    ptp = pctps.tile([128, 128], BF16, tag="tp")
                    nc.tensor.transpose(ptp, af2[:, kc, :], ident128)
                    nc.scalar.copy(aht[:, kc, :], ptp)
                p_giz1, p_in1 = s_gi_part(aht, "1")
                s1_b = pc1.tile([NFRAMES, UPC], BF16)
                s_gru_elem(p_giz1, gh_rz1, p_in1, gh_hn1, sc4s_sb, s1_b)
                # transpose s1 slice -> [units, frames], gather to full s1^T
                s1T = pc1.tile([128, 2, 128], BF16)
                for u2 in range(2):
                    ptp = pctps.tile([128, 128], BF16, tag="tp")
                    nc.tensor.transpose(ptp, s1_b[:, u2 * 128:(u2 + 1) * 128], ident128)
                    nc.scalar.copy(s1T[:, u2, :], ptp)
                with nc.named_scope("s1_ag"):
                    nc.scalar.dma_start(out=s1_cin.rearrange("(c p) n -> p c n", p=128), in_=s1T)
                    nc.gpsimd.collective_compute(
                        "AllGather", ALU.bypass, replica_groups=RG,
                        ins=[s1_cin.opt()], outs=[s1_cout.opt()])
                s1t_all = pc1.tile([128, KC, 128], BF16)
                nc.sync.dma_start(out=s1t_all, in_=s1_cout.rearrange("(kc p) n -> p kc n", p=128))
                p_giz2, p_in2 = s_gi_part(s1t_all, "2")
                out_sb = pc1.tile([NFRAMES, UPC], F32)
                s_gru_elem(p_giz2, gh_rz2, p_in2, gh_hn2, sfs_sb, out_sb)
                nc.sync.dma_start(out=outp.ap(), in_=out_sb)

    nc.compile()
    return nc


def _pm(a):
    """[KC'*128, N] -> partition-major [128, KC', N] contiguous."""
    rows, n = a.shape
    kc = rows // 128
    return np.ascontiguousarray(a.reshape(kc, 128, n).transpose(1, 0, 2))


def _blk_interleave(Wt):
    """[D, 3*D] gate-major -> [4, 128, KC, 1536] block-interleaved pm."""
    blocks = []
    for j in range(4):
        cols = [Wt[:, g * D + j * 512:g * D + (j + 1) * 512] for g in range(3)]
        blocks.append(_pm(np.ascontiguousarray(np.concatenate(cols, axis=1))))
    return np.stack(blocks)


def _blk_bias(b, scale):
    parts = [b[g * D + j * 512:g * D + (j + 1) * 512] for j in range(4) for g in range(3)]
    return (np.concatenate(parts)[None, :] * scale).astype(NB)


def _prep_in_maps(inputs):
    E = np.ascontiguousarray(inputs["H_O_edges"].reshape(NFRAMES, ROWS, D))
    On = inputs["O_nodes"].reshape(NFRAMES, O, D)
    Hn = inputs["H_nodes"].reshape(NFRAMES, H, D)
    Sc4 = inputs["S_node_C4"].reshape(NFRAMES, D)
    Sf = np.ascontiguousarray(inputs["final_S_node"].transpose(0, 2, 1)).reshape(NFRAMES, D)

    whi_t = np.ascontiguousarray(inputs["gh_wih"].T)
    whh_t = np.ascontiguousarray(inputs["gh_whh"].T)
    wsi_t = np.ascontiguousarray(inputs["gs_wih"].T)
    wsh_t = np.ascontiguousarray(inputs["gs_whh"].T)

    def slice_gates(Wt, c):
        return np.ascontiguousarray(np.concatenate(
            [Wt[:, g * D + c * UPC:g * D + (c + 1) * UPC] for g in range(3)], axis=1))

    def slice_bias(b, c):
        return np.concatenate(
            [b[g * D + c * UPC:g * D + (c + 1) * UPC] for g in range(3)])[None, :]

    # scat2[:, par, :]: maps quad-column (f,h,o) to Q-pair row (f + 4*par, o)
    # value 16 so wscat = scat*w = 16*w matches the fp8 scale plan
    scat = np.zeros((128, 2, 512), np.float32)
    for par in range(2):
        for f in range(4):
            for h in range(H):
                for o in range(O):
                    scat[(f + 4 * par) * O + o, par, f * 128 + h * O + o] = 16.0

    wnt_full = np.ascontiguousarray(inputs["Wn"].T).astype(NB)  # [D, 1024]
    wnt_q = np.stack([_pm(wnt_full[:, qr * 256:(qr + 1) * 256]) for qr in range(4)])

    shared = {
        "wcat": _pm(np.clip(np.ascontiguousarray(
            np.concatenate([inputs["We"], inputs["Wl1"]], axis=0).T) * 32.0,
            -240, 240).astype(N8)),
        "bl1t": np.ascontiguousarray(inputs["bl1"].reshape(8, 128).T).astype(np.float32),
        "bet0": np.ascontiguousarray(32.0 * inputs["be"].reshape(8, 128).T).astype(np.float32),
        "bet1": np.ascontiguousarray(512.0 * inputs["be"].reshape(8, 128).T).astype(np.float32),
        "pmat": np.ascontiguousarray(np.kron(np.eye(FPC), np.ones((H, 1))) / H).astype(NB),
        "wnt": wnt_q,
        "wnb": inputs["bn"][None, :].astype(NB),
        "wl2": np.ascontiguousarray(inputs["Wl2"][0].reshape(8, 128).T)[:, :, None].astype(NB),
        "scat2": scat.astype(NB),
        "whi_blk": np.clip(_blk_interleave(whi_t) * 32.0, -240, 240).astype(N8),
        "whh_blk": np.clip(_blk_interleave(whh_t) * 32.0, -240, 240).astype(N8),
        "bhi_r": _blk_bias(inputs["gh_bih"], 32768.0),
        "bhh_r": _blk_bias(inputs["gh_bhh"], 32.0),
        "sc4t": _pm(np.ascontiguousarray(Sc4.T).astype(NB)),
        "sft": _pm(np.ascontiguousarray(Sf.T).astype(NB)),
    }

    in_maps = []
    for c in range(NCORES):
        fr = slice(c * FPC, (c + 1) * FPC)
        us = slice(c * UPC, (c + 1) * UPC)
        Ec = E[fr]  # [16, 128, 2048]
        e0t = np.clip(np.ascontiguousarray(
            Ec.reshape(NQ, 4, ROWS, D).transpose(0, 3, 1, 2).reshape(NQ, D, 512)), -240, 240).astype(N8)
        e0t = np.ascontiguousarray(
            e0t.reshape(NQ, KC, 128, 512).transpose(0, 2, 1, 3))  # [NQ,128,KC,512]
        ot = _pm(np.ascontiguousarray(On[fr].reshape(FPC * O, D).T).astype(NB))
        Hl = Hn[fr].reshape(FPC * H, D)  # [128 local rows, D], row=(f,h)
        m = dict(shared)
        m.update({
            "e0t": e0t,
            "ot": ot,
            "htloc": _pm(np.clip(np.ascontiguousarray(Hl.T), -240, 240).astype(N8)),
            "hloc": Hl.astype(NB),
            "wsi_s": _pm(slice_gates(wsi_t, c).astype(NB)),
            "wsh_s": _pm(slice_gates(wsh_t, c).astype(NB)),
            "bsi_s": slice_bias(inputs["gs_bih"], c).astype(NB),
            "bsh_s": slice_bias(inputs["gs_bhh"], c).astype(NB),
            "sc4_s": np.ascontiguousarray(Sc4[:, us]).astype(np.float32),
            "sf_s": np.ascontiguousarray(Sf[:, us]).astype(np.float32),
        })
        in_maps.append(m)
    return in_maps


LAST_RESULT = None


def kernel(**inputs):
    global LAST_RESULT
    if "nc" not in _CACHE:
        _CACHE["nc"] = _build_nc()
    nc = _CACHE["nc"]
    in_maps = _prep_in_maps(inputs)
    trace = os.environ.get("KERNEL_TRACE", "0") == "1"
    res = bass_utils.run_bass_kernel_spmd(
        nc, in_maps, core_ids=list(range(NCORES)), trace=trace)
    LAST_RESULT = res
    out = np.concatenate([res.results[c]["outp"] for c in range(NCORES)], axis=1)
    return np.ascontiguousarray(out.reshape(B, F, D)).astype(np.float32)


if __name__ == "__main__":
    np.random.seed(0)
    ins = {
        "S_node_C4": np.random.randn(B, F, D).astype(np.float32),
        "final_S_node": np.random.randn(B, D, F).astype(np.float32),
        "H_nodes": np.random.randn(B, F, H, D).astype(np.float32),
        "O_nodes": np.random.randn(B, F, O, D).astype(np.float32),
        "H_O_edges": np.random.randn(B, F, H, O, D).astype(np.float32),
        "Wn": np.random.randn(D // 2, D).astype(np.float32) * 0.02,
        "bn": np.random.randn(D // 2).astype(np.float32) * 0.02,
        "We": np.random.randn(D // 2, D).astype(np.float32) * 0.02,
        "be": np.random.randn(D // 2).astype(np.float32) * 0.02,
        "Wl1": np.random.randn(D // 2, D).astype(np.float32) * 0.02,
        "bl1": np.random.randn(D // 2).astype(np.float32) * 0.02,
        "Wl2": np.random.randn(1, D // 2).astype(np.float32) * 0.02,
        "bl2": np.random.randn(1).astype(np.float32) * 0.02,
        "gh_wih": np.random.randn(3 * D, D).astype(np.float32) * 0.02,
        "gh_whh": np.random.randn(3 * D, D).astype(np.float32) * 0.02,
        "gh_bih": np.random.randn(3 * D).astype(np.float32) * 0.02,
        "gh_bhh": np.random.randn(3 * D).astype(np.float32) * 0.02,
        "gs_wih": np.random.randn(3 * D, D).astype(np.float32) * 0.02,
        "gs_whh": np.random.randn(3 * D, D).astype(np.float32) * 0.02,
        "gs_bih": np.random.randn(3 * D).astype(np.float32) * 0.02,
        "gs_bhh": np.random.randn(3 * D).astype(np.float32) * 0.02,
    }
    out = kernel(**ins)
    print("kernel ran, out shape", out.shape, out.dtype, float(np.abs(out).mean()))
